# revision 5
# baseline (speedup 1.0000x reference)
"""DGI (Deep Graph Infomax) forward on 8 Trainium2 NeuronCores.

Strategy (matches the sharding hint):
- Nodes are relabeled into 784 degree-balanced blocks of 128 (snake assignment
  by degree) and sharded 98 blocks/core. Edges partitioned by destination row.
- Phase A per core: h = relu(AX @ Wr^T) for both AX1/AX2 fused -> h_cat
  [12544, 512] bf16 (h1|h2 per row), AllGather -> full table [100352, 512].
- SpMM per 128-row block: dma_gather (int16, 4 col-groups of 25088 rows)
  fetches 1KB combined rows; DVE builds val-weighted one-hot S [128e,128r];
  PE matmul (lhsT=S, rhs=G) accumulates agg for both passes in one PSUM bank.
- Second GCN matmul via PE transposes; masked ones-matmul accumulates the
  readout sum; AllReduce + sigmoid + bilinear scores on device.
- Host scatters per-core sample scores back to the [1, 2*8192] output.
"""
import numpy as np
import ml_dtypes

from concourse import bass, bacc, mybir, tile
from concourse.bass_utils import run_bass_kernel_spmd

# problem constants (hardcoded per contract)
N = 100000
E = 3200000
NIN = 512
NH = 256
NSAMP = 8192
P = 128
NCORES = 8
NBLK = 784            # total 128-row blocks
NPB = NBLK // NCORES  # 98 blocks per core
NLOC = NPB * P        # 12544 rows per core
NPAD = NBLK * P       # 100352 padded rows
NGRP = 4
VG = NPAD // NGRP     # 25088 rows per int16-addressable gather group
DT = mybir.dt
F32 = DT.float32
BF16 = DT.bfloat16

_PROG_CACHE = {}


def _build_program(TG, NSB):
    """TG: gather tiles per (block, group); NSB: sample tiles per core."""
    NIB = TG * P          # idxs per dma_gather call
    TPB = NGRP * TG       # edge tiles per block
    nc = bacc.Bacc("TRN2", target_bir_lowering=False, debug=False,
                   num_devices=NCORES)

    axt1 = nc.dram_tensor("axt1", [NIN, NLOC], F32, kind="ExternalInput")
    axt2 = nc.dram_tensor("axt2", [NIN, NLOC], F32, kind="ExternalInput")
    wrt = nc.dram_tensor("wrt", [NIN, NH], F32, kind="ExternalInput")
    wt = nc.dram_tensor("wt", [NH, NH], F32, kind="ExternalInput")
    wb = nc.dram_tensor("wb", [NH, NH], F32, kind="ExternalInput")
    bilb = nc.dram_tensor("bilb", [1, 1], F32, kind="ExternalInput")
    iota = nc.dram_tensor("iota", [P, P], BF16, kind="ExternalInput")
    ident = nc.dram_tensor("ident", [P, P], F32, kind="ExternalInput")
    idxg = nc.dram_tensor("idxg", [NPB, P, NGRP * (NIB // 16)], DT.int16,
                          kind="ExternalInput")
    rla = nc.dram_tensor("rla", [P, NPB * TPB], F32, kind="ExternalInput")
    vla = nc.dram_tensor("vla", [P, NPB * TPB], F32, kind="ExternalInput")
    mska = nc.dram_tensor("mska", [P, NPB], F32, kind="ExternalInput")
    sidx = nc.dram_tensor("sidx", [P, NSB], DT.int32, kind="ExternalInput")
    out_sc = nc.dram_tensor("out_sc", [2, NSB * P], F32, kind="ExternalOutput")

    with tile.TileContext(nc) as tc:
        with (
            tc.tile_pool(name="dram", bufs=1, space="DRAM") as dpool,
            tc.tile_pool(name="const", bufs=1) as cpool,
            tc.tile_pool(name="work", bufs=2) as wpool,
            tc.tile_pool(name="spool", bufs=3) as spool,
            tc.tile_pool(name="psA", bufs=2, space="PSUM") as psA_p,
            tc.tile_pool(name="psB", bufs=2, space="PSUM") as psB_p,
            tc.tile_pool(name="psT", bufs=2, space="PSUM") as psT_p,
            tc.tile_pool(name="psM", bufs=1, space="PSUM") as psM_p,
            tc.tile_pool(name="psY", bufs=1, space="PSUM") as psY_p,
        ):
            hcat_loc = dpool.tile([NLOC, 2 * NH], BF16)
            hcat_full = dpool.tile([NPAD, 2 * NH], BF16)
            h12f_dram = dpool.tile([NLOC, 2 * NH], F32)
            mr_in = dpool.tile([1, 2 * NH], F32)
            mr_out = dpool.tile([1, 2 * NH], F32)

            # constants
            wrt_sb = cpool.tile([P, 4 * NH], F32)
            for c in range(4):
                nc.sync.dma_start(out=wrt_sb[:, c * NH:(c + 1) * NH],
                                  in_=wrt[c * P:(c + 1) * P, :])
            wt_sb = cpool.tile([P, 2 * NH], F32)
            for c in range(2):
                nc.sync.dma_start(out=wt_sb[:, c * NH:(c + 1) * NH],
                                  in_=wt[c * P:(c + 1) * P, :])
            wb_sb = cpool.tile([P, 2 * NH], F32)
            for c in range(2):
                nc.sync.dma_start(out=wb_sb[:, c * NH:(c + 1) * NH],
                                  in_=wb[c * P:(c + 1) * P, :])
            bilb_sb = cpool.tile([1, 1], F32)
            nc.sync.dma_start(out=bilb_sb[:], in_=bilb[:])
            iota_sb = cpool.tile([P, P], BF16)
            nc.sync.dma_start(out=iota_sb[:], in_=iota[:])
            ident_sb = cpool.tile([P, P], F32)
            nc.sync.dma_start(out=ident_sb[:], in_=ident[:])
            ones_sb = cpool.tile([1, 1], F32)
            nc.vector.memset(ones_sb[:], 1.0)
            rl_sb = cpool.tile([P, NPB * TPB], F32)
            nc.sync.dma_start(out=rl_sb[:], in_=rla[:])
            vl_sb = cpool.tile([P, NPB * TPB], F32)
            nc.sync.dma_start(out=vl_sb[:], in_=vla[:])
            msk_sb = cpool.tile([P, NPB], F32)
            nc.sync.dma_start(out=msk_sb[:], in_=mska[:])
            sidx_sb = cpool.tile([P, NSB], DT.int32)
            nc.sync.dma_start(out=sidx_sb[:], in_=sidx[:])

            # ---- phase A: h_cat = relu(AX @ Wr^T) for both passes ----
            for b in range(NPB):
                psA = psA_p.tile([P, 2 * NH], F32, tag="mmA")
                for p, axt in enumerate((axt1, axt2)):
                    for c in range(4):
                        ax_t = wpool.tile([P, P], F32, tag="axt")
                        nc.sync.dma_start(
                            out=ax_t[:],
                            in_=axt[c * P:(c + 1) * P, b * P:(b + 1) * P])
                        nc.tensor.matmul(
                            out=psA[:, p * NH:(p + 1) * NH],
                            lhsT=ax_t[:],
                            rhs=wrt_sb[:, c * NH:(c + 1) * NH],
                            start=(c == 0), stop=(c == 3))
                hc_t = wpool.tile([P, 2 * NH], BF16, tag="hcat")
                nc.scalar.activation(hc_t[:], psA[:],
                                     mybir.ActivationFunctionType.Relu)
                nc.scalar.dma_start(out=hcat_loc[b * P:(b + 1) * P, :],
                                    in_=hc_t[:])

            # ---- AllGather h_cat ----
            nc.gpsimd.collective_compute(
                "AllGather", mybir.AluOpType.bypass,
                replica_groups=[list(range(NCORES))],
                ins=[hcat_loc.opt()], outs=[hcat_full.opt()])

            # ---- phase B (SpMM) + phase C per block ----
            psM = psM_p.tile([1, 2 * NH], F32)
            for b in range(NPB):
                idx_t = wpool.tile([P, NGRP * (NIB // 16)], DT.int16, tag="idx")
                nc.sync.dma_start(out=idx_t[:], in_=idxg[b])
                G = wpool.tile([P, TPB * 2 * NH], BF16, tag="G")
                for g in range(NGRP):
                    nc.gpsimd.dma_gather(
                        G[:, g * TG * 2 * NH:(g + 1) * TG * 2 * NH].rearrange(
                            "p (t d) -> p t d", d=2 * NH),
                        hcat_full[g * VG:(g + 1) * VG, :],
                        idx_t[:, g * (NIB // 16):(g + 1) * (NIB // 16)],
                        NIB, NIB, 2 * NH, single_packet=False)
                psB = psB_p.tile([P, 2 * NH], F32, tag="mmB")
                for t in range(TPB):
                    col = b * TPB + t
                    S = spool.tile([P, P], BF16, tag="S")
                    nc.vector.tensor_scalar(
                        out=S[:], in0=iota_sb[:],
                        scalar1=rl_sb[:, col:col + 1],
                        scalar2=vl_sb[:, col:col + 1],
                        op0=mybir.AluOpType.is_equal,
                        op1=mybir.AluOpType.mult)
                    nc.tensor.matmul(
                        out=psB[:], lhsT=S[:],
                        rhs=G[:, t * 2 * NH:(t + 1) * 2 * NH],
                        start=(t == 0), stop=(t == TPB - 1))
                aggS = wpool.tile([P, 2 * NH], F32, tag="aggS")
                nc.vector.tensor_copy(aggS[:], psB[:])
                psT = psT_p.tile([P, 2 * NH], F32, tag="mmT")
                for c in range(4):
                    nc.tensor.transpose(
                        out=psT[:, c * P:(c + 1) * P],
                        in_=aggS[:, c * P:(c + 1) * P],
                        identity=ident_sb[:])
                aggT = wpool.tile([P, 2 * NH], F32, tag="aggT")
                nc.vector.tensor_copy(aggT[:], psT[:])
                psC = psA_p.tile([P, 2 * NH], F32, tag="mmA")
                for p in range(2):
                    for c2 in range(2):
                        nc.tensor.matmul(
                            out=psC[:, p * NH:(p + 1) * NH],
                            lhsT=aggT[:, (p * 2 + c2) * P:(p * 2 + c2 + 1) * P],
                            rhs=wt_sb[:, c2 * NH:(c2 + 1) * NH],
                            start=(c2 == 0), stop=(c2 == 1))
                h12 = wpool.tile([P, 2 * NH], F32, tag="h12")
                nc.scalar.activation(h12[:], psC[:],
                                     mybir.ActivationFunctionType.Relu)
                nc.scalar.dma_start(out=h12f_dram[b * P:(b + 1) * P, :],
                                    in_=h12[:])
                nc.tensor.matmul(out=psM[:], lhsT=msk_sb[:, b:b + 1],
                                 rhs=h12[:], start=(b == 0), stop=(b == NPB - 1))

            # ---- readout mean -> AllReduce -> c -> cW ----
            msum = wpool.tile([1, 2 * NH], F32, tag="msum")
            nc.vector.tensor_copy(msum[:], psM[:])
            nc.scalar.dma_start(out=mr_in[:], in_=msum[:])
            nc.gpsimd.collective_compute(
                "AllReduce", mybir.AluOpType.add,
                replica_groups=[list(range(NCORES))],
                ins=[mr_in.opt()], outs=[mr_out.opt()])
            msum2 = wpool.tile([1, 2 * NH], F32, tag="msum2")
            nc.sync.dma_start(out=msum2[:], in_=mr_out[:])
            # transpose pass-1 sums into [128,1] chunks via k=1 matmul
            psct = psY_p.tile([P, 2], F32, tag="tiny")
            for c in range(2):
                nc.tensor.matmul(out=psct[:, c:c + 1],
                                 lhsT=msum2[:1, c * P:(c + 1) * P],
                                 rhs=ones_sb[:1, :1], start=True, stop=True)
            ct_sb = wpool.tile([P, 2], F32, tag="ct")
            nc.scalar.activation(ct_sb[:], psct[:],
                                 mybir.ActivationFunctionType.Sigmoid,
                                 scale=1.0 / N)
            psW = psY_p.tile([1, NH], F32, tag="tiny")
            for c in range(2):
                nc.tensor.matmul(out=psW[:], lhsT=ct_sb[:, c:c + 1],
                                 rhs=wb_sb[:, c * NH:(c + 1) * NH],
                                 start=(c == 0), stop=(c == 1))
            cw_sb = wpool.tile([1, NH], F32, tag="cw")
            nc.vector.tensor_copy(cw_sb[:], psW[:])
            psct2 = psY_p.tile([P, 2], F32, tag="tiny")
            for c in range(2):
                nc.tensor.matmul(out=psct2[:, c:c + 1],
                                 lhsT=cw_sb[:1, c * P:(c + 1) * P],
                                 rhs=ones_sb[:1, :1], start=True, stop=True)
            cwt_sb = wpool.tile([P, 2], F32, tag="cwt")
            nc.vector.tensor_copy(cwt_sb[:], psct2[:])

            # ---- scores for sampled nodes ----
            for st in range(NSB):
                hs = wpool.tile([P, 2 * NH], F32, tag="hs")
                nc.gpsimd.indirect_dma_start(
                    out=hs[:], out_offset=None, in_=h12f_dram[:],
                    in_offset=bass.IndirectOffsetOnAxis(
                        ap=sidx_sb[:, st:st + 1], axis=0))
                psS = psT_p.tile([P, 2 * NH], F32, tag="mmT")
                for c in range(4):
                    nc.tensor.transpose(out=psS[:, c * P:(c + 1) * P],
                                        in_=hs[:, c * P:(c + 1) * P],
                                        identity=ident_sb[:])
                hsT = wpool.tile([P, 2 * NH], F32, tag="hsT")
                nc.vector.tensor_copy(hsT[:], psS[:])
                for p in range(2):
                    psSc = psY_p.tile([1, P], F32, tag="tiny")
                    for c in range(2):
                        nc.tensor.matmul(
                            out=psSc[:],
                            lhsT=cwt_sb[:, c:c + 1],
                            rhs=hsT[:, (p * 2 + c) * P:(p * 2 + c + 1) * P],
                            start=(c == 0), stop=(c == 1))
                    sc_sb = wpool.tile([1, P], F32, tag="scsb")
                    nc.vector.tensor_scalar(
                        out=sc_sb[:], in0=psSc[:], scalar1=bilb_sb[:1, :1],
                        scalar2=None, op0=mybir.AluOpType.add)
                    nc.scalar.dma_start(
                        out=out_sc[p:p + 1, st * P:(st + 1) * P],
                        in_=sc_sb[:])

    nc.compile()
    return nc


def _prep(inputs):
    """Host-side sharding/permutation. Returns (in_maps, meta)."""
    er = np.asarray(inputs["edge_rows"]).astype(np.int64)
    ec = np.asarray(inputs["edge_cols"]).astype(np.int64)
    ev = np.asarray(inputs["edge_vals"]).astype(np.float32)
    AX1 = np.asarray(inputs["AX1"], dtype=np.float32)
    AX2 = np.asarray(inputs["AX2"], dtype=np.float32)
    nodes = np.asarray(inputs["nodes"]).astype(np.int64)
    Wr_w = np.asarray(inputs["Wr_w"], dtype=np.float32)
    W_w = np.asarray(inputs["W_w"], dtype=np.float32)
    bil_W = np.asarray(inputs["bil_W"], dtype=np.float32)
    bil_b = np.asarray(inputs["bil_b"], dtype=np.float32)

    # --- degree-balanced node relabeling (snake over sorted degrees) ---
    deg = np.bincount(er, minlength=N)
    order = np.argsort(-deg, kind="stable")          # node ids, desc degree
    rank = np.empty(N, dtype=np.int64)
    rank[order] = np.arange(N)
    rnd = rank // NBLK                                # round = slot
    pos = rank % NBLK
    blk = np.where(rnd % 2 == 0, pos, NBLK - 1 - pos)
    new_id = blk * P + rnd                            # [N]
    old_of_new = np.full(NPAD, -1, dtype=np.int64)
    old_of_new[new_id] = np.arange(N)

    rows_n = new_id[er]
    cols_n = new_id[ec]

    # --- per-core edge buckets ---
    core = rows_n // NLOC
    TG = 0
    per_core = []
    for d in range(NCORES):
        sel = np.nonzero(core == d)[0]
        r = rows_n[sel] - d * NLOC
        c = cols_n[sel]
        v = ev[sel]
        bl = r // P
        rloc = r % P
        grp = c // VG
        colg = (c % VG).astype(np.int16)
        key = bl * NGRP + grp
        o = np.argsort(key, kind="stable")
        key = key[o]; rloc = rloc[o]; colg = colg[o]; v = v[o]
        cnt = np.bincount(key, minlength=NPB * NGRP)
        TG = max(TG, int((cnt.max() + P - 1) // P))
        per_core.append((key, rloc, colg, v, cnt))

    NIB = TG * P
    TPB = NGRP * TG

    # --- samples ---
    snew = new_id[nodes]
    sowner = snew // NLOC
    slocal = snew % NLOC
    scnt = np.bincount(sowner, minlength=NCORES)
    NSB = max(1, int((scnt.max() + P - 1) // P))
    NS = NSB * P

    iota_np = np.tile(np.arange(P, dtype=np.float32)[None, :], (P, 1)).astype(
        ml_dtypes.bfloat16)
    ident_np = np.eye(P, dtype=np.float32)
    wrt_np = np.ascontiguousarray(Wr_w.T)            # [512, 256]
    wt_np = np.ascontiguousarray(W_w.T)              # [256, 256]
    wb_np = np.ascontiguousarray(bil_W[0])           # [256, 256]
    bilb_np = bil_b.reshape(1, 1)

    in_maps = []
    sample_pos = []   # (core, position-in-core-order) per original sample
    for d in range(NCORES):
        key, rloc, colg, v, cnt = per_core[d]
        start = np.concatenate([[0], np.cumsum(cnt)])[:-1]
        off = np.arange(len(key)) - start[key]
        bl = key // NGRP
        grp = key % NGRP
        tl = grp * TG + off // P                      # tile within block
        lane = off % P
        idx_arr = np.zeros((NPB, NGRP, NIB), dtype=np.int16)
        idx_arr[bl, grp, off] = colg
        rl_arr = np.zeros((NPB, P, TPB), dtype=np.float32)
        vl_arr = np.zeros((NPB, P, TPB), dtype=np.float32)
        rl_arr[bl, lane, tl] = rloc
        vl_arr[bl, lane, tl] = v
        # wrapped idx layout: value k -> partition k%16 (x8 replicated), col k//16
        w = idx_arr.reshape(NPB, NGRP, NIB // 16, 16).transpose(0, 1, 3, 2)
        w = np.tile(w, (1, 1, 8, 1))                 # [NPB, NGRP, 128, NIB//16]
        idxg_np = np.ascontiguousarray(
            w.transpose(0, 2, 1, 3).reshape(NPB, P, NGRP * (NIB // 16)))
        rla_np = np.ascontiguousarray(
            rl_arr.transpose(1, 0, 2).reshape(P, NPB * TPB))
        vla_np = np.ascontiguousarray(
            vl_arr.transpose(1, 0, 2).reshape(P, NPB * TPB))
        # mask of occupied slots
        occ = (old_of_new[d * NLOC:(d + 1) * NLOC] >= 0).astype(np.float32)
        mska_np = np.ascontiguousarray(occ.reshape(NPB, P).T)   # [P, NPB]
        # AXT shards
        olds = old_of_new[d * NLOC:(d + 1) * NLOC]
        valid = olds >= 0
        tmp1 = np.zeros((NLOC, NIN), dtype=np.float32)
        tmp1[valid] = AX1[olds[valid]]
        tmp2 = np.zeros((NLOC, NIN), dtype=np.float32)
        tmp2[valid] = AX2[olds[valid]]
        axt1_np = np.ascontiguousarray(tmp1.T)
        axt2_np = np.ascontiguousarray(tmp2.T)
        # samples owned by this core
        sp = np.nonzero(sowner == d)[0]
        sample_pos.append(sp)
        sl = np.zeros(NS, dtype=np.int32)
        sl[:len(sp)] = slocal[sp]
        sidx_np = np.ascontiguousarray(sl.reshape(NSB, P).T)    # [P, NSB]
        in_maps.append(dict(
            axt1=axt1_np, axt2=axt2_np, wrt=wrt_np, wt=wt_np, wb=wb_np,
            bilb=bilb_np, iota=iota_np, ident=ident_np, idxg=idxg_np,
            rla=rla_np, vla=vla_np, mska=mska_np, sidx=sidx_np))
    return in_maps, (TG, NSB, sample_pos)


def kernel(**inputs) -> np.ndarray:
    in_maps, (TG, NSB, sample_pos) = _prep(inputs)
    key = (TG, NSB)
    if key not in _PROG_CACHE:
        _PROG_CACHE[key] = _build_program(TG, NSB)
    nc = _PROG_CACHE[key]
    last = None
    for _ in range(3):
        try:
            res = run_bass_kernel_spmd(nc, in_maps, core_ids=list(range(NCORES)))
            break
        except Exception as e:   # wedged device -> retry
            last = e
    else:
        raise last
    out = np.zeros((1, 2 * NSAMP), dtype=np.float32)
    for d in range(NCORES):
        sc = res.results[d]["out_sc"]          # [2, NS]
        sp = sample_pos[d]
        out[0, sp] = sc[0, :len(sp)]
        out[0, NSAMP + sp] = sc[1, :len(sp)]
    return out


# revision 6
# speedup vs baseline: 1.3837x; 1.3837x over previous
"""DGI (Deep Graph Infomax) forward on 8 Trainium2 NeuronCores.

Strategy (matches the sharding hint):
- Nodes are relabeled into 784 degree-balanced blocks of 128 (snake assignment
  by degree) and sharded 98 blocks/core. Edges partitioned by destination row.
- Phase A per core: h = relu(AX @ Wr^T) for both AX1/AX2 fused -> h_cat
  [12544, 512] bf16 (h1|h2 per row), AllGather -> full table [100352, 512].
- SpMM per 128-row block: dma_gather (int16, 4 col-groups of 25088 rows)
  fetches 1KB combined rows; DVE builds val-weighted one-hot S [128e,128r];
  PE matmul (lhsT=S, rhs=G) accumulates agg for both passes in one PSUM bank.
- Second GCN matmul via PE transposes; masked ones-matmul accumulates the
  readout sum; AllReduce + sigmoid + bilinear scores on device.
- Host scatters per-core sample scores back to the [1, 2*8192] output.
"""
import numpy as np
import ml_dtypes

from concourse import bass, bacc, mybir, tile
from concourse.bass_utils import run_bass_kernel_spmd

# problem constants (hardcoded per contract)
N = 100000
E = 3200000
NIN = 512
NH = 256
NSAMP = 8192
P = 128
NCORES = 8
NBLK = 784            # total 128-row blocks
NPB = NBLK // NCORES  # 98 blocks per core
NLOC = NPB * P        # 12544 rows per core
NPAD = NBLK * P       # 100352 padded rows
NGRP = 4
VG = NPAD // NGRP     # 25088 rows per int16-addressable gather group
DT = mybir.dt
F32 = DT.float32
BF16 = DT.bfloat16

_PROG_CACHE = {}


def _build_program(TG, NSB):
    """TG: gather tiles per (block, group); NSB: sample tiles per core."""
    NIB = TG * P          # idxs per dma_gather call
    TPB = NGRP * TG       # edge tiles per block
    nc = bacc.Bacc("TRN2", target_bir_lowering=False, debug=False,
                   num_devices=NCORES)

    axt1 = nc.dram_tensor("axt1", [NIN, NLOC], BF16, kind="ExternalInput")
    axt2 = nc.dram_tensor("axt2", [NIN, NLOC], BF16, kind="ExternalInput")
    wrt = nc.dram_tensor("wrt", [NIN, NH], BF16, kind="ExternalInput")
    wt = nc.dram_tensor("wt", [NH, NH], BF16, kind="ExternalInput")
    wb = nc.dram_tensor("wb", [NH, NH], F32, kind="ExternalInput")
    bilb = nc.dram_tensor("bilb", [1, 1], F32, kind="ExternalInput")
    ident = nc.dram_tensor("ident", [P, P], F32, kind="ExternalInput")
    idxg = nc.dram_tensor("idxg", [NPB, P, NGRP * (NIB // 16)], DT.int16,
                          kind="ExternalInput")
    sva = nc.dram_tensor("sva", [NPB, P, TPB * P], BF16, kind="ExternalInput")
    mska = nc.dram_tensor("mska", [P, NPB], F32, kind="ExternalInput")
    sidx = nc.dram_tensor("sidx", [P, NSB], DT.int32, kind="ExternalInput")
    out_sc = nc.dram_tensor("out_sc", [2, NSB * P], F32, kind="ExternalOutput")

    with tile.TileContext(nc) as tc:
        with (
            tc.tile_pool(name="dram", bufs=1, space="DRAM") as dpool,
            tc.tile_pool(name="const", bufs=1) as cpool,
            tc.tile_pool(name="work", bufs=2) as wpool,
            tc.tile_pool(name="spool", bufs=3) as spool,
            tc.tile_pool(name="psA", bufs=2, space="PSUM") as psA_p,
            tc.tile_pool(name="psB", bufs=2, space="PSUM") as psB_p,
            tc.tile_pool(name="psT", bufs=2, space="PSUM") as psT_p,
            tc.tile_pool(name="psM", bufs=1, space="PSUM") as psM_p,
            tc.tile_pool(name="psY", bufs=1, space="PSUM") as psY_p,
        ):
            hcat_loc = dpool.tile([NLOC, 2 * NH], BF16)
            hcat_full = nc.dram_tensor("hcat_full", [NPAD, 2 * NH], BF16,
                                       addr_space="Shared")
            h12f_dram = dpool.tile([NLOC, 2 * NH], F32)
            mr_in = dpool.tile([1, 2 * NH], F32)
            mr_out = nc.dram_tensor("mr_out", [1, 2 * NH], F32,
                                    addr_space="Shared")

            # constants
            wrt_sb = cpool.tile([P, 4 * NH], BF16)
            for c in range(4):
                nc.sync.dma_start(out=wrt_sb[:, c * NH:(c + 1) * NH],
                                  in_=wrt[c * P:(c + 1) * P, :])
            wt_sb = cpool.tile([P, 2 * NH], BF16)
            for c in range(2):
                nc.sync.dma_start(out=wt_sb[:, c * NH:(c + 1) * NH],
                                  in_=wt[c * P:(c + 1) * P, :])
            wb_sb = cpool.tile([P, 2 * NH], F32)
            for c in range(2):
                nc.sync.dma_start(out=wb_sb[:, c * NH:(c + 1) * NH],
                                  in_=wb[c * P:(c + 1) * P, :])
            bilb_sb = cpool.tile([1, 1], F32)
            nc.sync.dma_start(out=bilb_sb[:], in_=bilb[:])
            ident_sb = cpool.tile([P, P], F32)
            nc.sync.dma_start(out=ident_sb[:], in_=ident[:])
            ones_sb = cpool.tile([1, 1], F32)
            nc.vector.memset(ones_sb[:], 1.0)
            msk_sb = cpool.tile([P, NPB], F32)
            nc.sync.dma_start(out=msk_sb[:], in_=mska[:])
            sidx_sb = cpool.tile([P, NSB], DT.int32)
            nc.sync.dma_start(out=sidx_sb[:], in_=sidx[:])

            # ---- phase A: h_cat = relu(AX @ Wr^T), two blocks at a time ----
            for bp in range(NPB // 2):
                ax_ts = {}
                for p, axt in enumerate((axt1, axt2)):
                    for c in range(4):
                        ax_t = wpool.tile([P, 2 * P], BF16, tag="axt")
                        nc.sync.dma_start(
                            out=ax_t[:],
                            in_=axt[c * P:(c + 1) * P,
                                    bp * 2 * P:(bp + 1) * 2 * P])
                        ax_ts[(p, c)] = ax_t
                for j in range(2):
                    b = 2 * bp + j
                    psA = psA_p.tile([P, 2 * NH], F32, tag="mmA")
                    for p in range(2):
                        for c in range(4):
                            nc.tensor.matmul(
                                out=psA[:, p * NH:(p + 1) * NH],
                                lhsT=ax_ts[(p, c)][:, j * P:(j + 1) * P],
                                rhs=wrt_sb[:, c * NH:(c + 1) * NH],
                                start=(c == 0), stop=(c == 3))
                    hc_t = wpool.tile([P, 2 * NH], BF16, tag="hcat")
                    nc.scalar.activation(hc_t[:], psA[:],
                                         mybir.ActivationFunctionType.Relu)
                    nc.scalar.dma_start(out=hcat_loc[b * P:(b + 1) * P, :],
                                        in_=hc_t[:])

            # ---- AllGather h_cat ----
            nc.gpsimd.collective_compute(
                "AllGather", mybir.AluOpType.bypass,
                replica_groups=[list(range(NCORES))],
                ins=[hcat_loc.opt()], outs=[hcat_full[:]])

            # ---- phase B (SpMM) + phase C per block ----
            psM = psM_p.tile([1, 2 * NH], F32)
            for b in range(NPB):
                idx_t = wpool.tile([P, NGRP * (NIB // 16)], DT.int16, tag="idx")
                nc.sync.dma_start(out=idx_t[:], in_=idxg[b])
                G = wpool.tile([P, TPB * 2 * NH], BF16, tag="G")
                for g in range(NGRP):
                    nc.gpsimd.dma_gather(
                        G[:, g * TG * 2 * NH:(g + 1) * TG * 2 * NH].rearrange(
                            "p (t d) -> p t d", d=2 * NH),
                        hcat_full[g * VG:(g + 1) * VG, :],
                        idx_t[:, g * (NIB // 16):(g + 1) * (NIB // 16)],
                        NIB, NIB, 2 * NH, single_packet=False)
                S_all = spool.tile([P, TPB * P], BF16, tag="S")
                nc.sync.dma_start(out=S_all[:], in_=sva[b])
                psB = psB_p.tile([P, 2 * NH], F32, tag="mmB")
                for t in range(TPB):
                    nc.tensor.matmul(
                        out=psB[:], lhsT=S_all[:, t * P:(t + 1) * P],
                        rhs=G[:, t * 2 * NH:(t + 1) * 2 * NH],
                        start=(t == 0), stop=(t == TPB - 1))
                aggS = wpool.tile([P, 2 * NH], F32, tag="aggS")
                nc.vector.tensor_copy(aggS[:], psB[:])
                psT = psT_p.tile([P, 2 * NH], F32, tag="mmT")
                for c in range(4):
                    nc.tensor.transpose(
                        out=psT[:, c * P:(c + 1) * P],
                        in_=aggS[:, c * P:(c + 1) * P],
                        identity=ident_sb[:])
                aggT = wpool.tile([P, 2 * NH], BF16, tag="aggT")
                nc.vector.tensor_copy(aggT[:], psT[:])
                psC = psA_p.tile([P, 2 * NH], F32, tag="mmA")
                for p in range(2):
                    for c2 in range(2):
                        nc.tensor.matmul(
                            out=psC[:, p * NH:(p + 1) * NH],
                            lhsT=aggT[:, (p * 2 + c2) * P:(p * 2 + c2 + 1) * P],
                            rhs=wt_sb[:, c2 * NH:(c2 + 1) * NH],
                            start=(c2 == 0), stop=(c2 == 1))
                h12 = wpool.tile([P, 2 * NH], F32, tag="h12")
                nc.scalar.activation(h12[:], psC[:],
                                     mybir.ActivationFunctionType.Relu)
                nc.scalar.dma_start(out=h12f_dram[b * P:(b + 1) * P, :],
                                    in_=h12[:])
                nc.tensor.matmul(out=psM[:], lhsT=msk_sb[:, b:b + 1],
                                 rhs=h12[:], start=(b == 0), stop=(b == NPB - 1))

            # ---- readout mean -> AllReduce -> c -> cW ----
            msum = wpool.tile([1, 2 * NH], F32, tag="msum")
            nc.vector.tensor_copy(msum[:], psM[:])
            nc.scalar.dma_start(out=mr_in[:], in_=msum[:])
            nc.gpsimd.collective_compute(
                "AllReduce", mybir.AluOpType.add,
                replica_groups=[list(range(NCORES))],
                ins=[mr_in.opt()], outs=[mr_out[:]])
            msum2 = wpool.tile([1, 2 * NH], F32, tag="msum2")
            nc.sync.dma_start(out=msum2[:], in_=mr_out[:])
            # transpose pass-1 sums into [128,1] chunks via k=1 matmul
            psct = psY_p.tile([P, 2], F32, tag="tiny")
            for c in range(2):
                nc.tensor.matmul(out=psct[:, c:c + 1],
                                 lhsT=msum2[:1, c * P:(c + 1) * P],
                                 rhs=ones_sb[:1, :1], start=True, stop=True)
            ct_sb = wpool.tile([P, 2], F32, tag="ct")
            nc.scalar.activation(ct_sb[:], psct[:],
                                 mybir.ActivationFunctionType.Sigmoid,
                                 scale=1.0 / N)
            psW = psY_p.tile([1, NH], F32, tag="tiny")
            for c in range(2):
                nc.tensor.matmul(out=psW[:], lhsT=ct_sb[:, c:c + 1],
                                 rhs=wb_sb[:, c * NH:(c + 1) * NH],
                                 start=(c == 0), stop=(c == 1))
            cw_sb = wpool.tile([1, NH], F32, tag="cw")
            nc.vector.tensor_copy(cw_sb[:], psW[:])
            psct2 = psY_p.tile([P, 2], F32, tag="tiny")
            for c in range(2):
                nc.tensor.matmul(out=psct2[:, c:c + 1],
                                 lhsT=cw_sb[:1, c * P:(c + 1) * P],
                                 rhs=ones_sb[:1, :1], start=True, stop=True)
            cwt_sb = wpool.tile([P, 2], F32, tag="cwt")
            nc.vector.tensor_copy(cwt_sb[:], psct2[:])

            # ---- scores for sampled nodes ----
            for st in range(NSB):
                hs = wpool.tile([P, 2 * NH], F32, tag="hs")
                nc.gpsimd.indirect_dma_start(
                    out=hs[:], out_offset=None, in_=h12f_dram[:],
                    in_offset=bass.IndirectOffsetOnAxis(
                        ap=sidx_sb[:, st:st + 1], axis=0))
                psS = psT_p.tile([P, 2 * NH], F32, tag="mmT")
                for c in range(4):
                    nc.tensor.transpose(out=psS[:, c * P:(c + 1) * P],
                                        in_=hs[:, c * P:(c + 1) * P],
                                        identity=ident_sb[:])
                hsT = wpool.tile([P, 2 * NH], F32, tag="hsT")
                nc.vector.tensor_copy(hsT[:], psS[:])
                for p in range(2):
                    psSc = psY_p.tile([1, P], F32, tag="tiny")
                    for c in range(2):
                        nc.tensor.matmul(
                            out=psSc[:],
                            lhsT=cwt_sb[:, c:c + 1],
                            rhs=hsT[:, (p * 2 + c) * P:(p * 2 + c + 1) * P],
                            start=(c == 0), stop=(c == 1))
                    sc_sb = wpool.tile([1, P], F32, tag="scsb")
                    nc.vector.tensor_scalar(
                        out=sc_sb[:], in0=psSc[:], scalar1=bilb_sb[:1, :1],
                        scalar2=None, op0=mybir.AluOpType.add)
                    nc.scalar.dma_start(
                        out=out_sc[p:p + 1, st * P:(st + 1) * P],
                        in_=sc_sb[:])

    nc.compile()
    return nc


def _prep(inputs):
    """Host-side sharding/permutation. Returns (in_maps, meta)."""
    er = np.asarray(inputs["edge_rows"]).astype(np.int64)
    ec = np.asarray(inputs["edge_cols"]).astype(np.int64)
    ev = np.asarray(inputs["edge_vals"]).astype(np.float32)
    AX1 = np.asarray(inputs["AX1"], dtype=np.float32)
    AX2 = np.asarray(inputs["AX2"], dtype=np.float32)
    nodes = np.asarray(inputs["nodes"]).astype(np.int64)
    Wr_w = np.asarray(inputs["Wr_w"], dtype=np.float32)
    W_w = np.asarray(inputs["W_w"], dtype=np.float32)
    bil_W = np.asarray(inputs["bil_W"], dtype=np.float32)
    bil_b = np.asarray(inputs["bil_b"], dtype=np.float32)

    # --- degree-balanced node relabeling (snake over sorted degrees) ---
    deg = np.bincount(er, minlength=N)
    order = np.argsort(-deg, kind="stable")          # node ids, desc degree
    rank = np.empty(N, dtype=np.int64)
    rank[order] = np.arange(N)
    rnd = rank // NBLK                                # round = slot
    pos = rank % NBLK
    blk = np.where(rnd % 2 == 0, pos, NBLK - 1 - pos)
    new_id = blk * P + rnd                            # [N]
    old_of_new = np.full(NPAD, -1, dtype=np.int64)
    old_of_new[new_id] = np.arange(N)

    rows_n = new_id[er]
    cols_n = new_id[ec]

    # --- per-core edge buckets ---
    core = rows_n // NLOC
    TG = 0
    per_core = []
    for d in range(NCORES):
        sel = np.nonzero(core == d)[0]
        r = rows_n[sel] - d * NLOC
        c = cols_n[sel]
        v = ev[sel]
        bl = r // P
        rloc = r % P
        grp = c // VG
        colg = (c % VG).astype(np.int16)
        key = bl * NGRP + grp
        o = np.argsort(key, kind="stable")
        key = key[o]; rloc = rloc[o]; colg = colg[o]; v = v[o]
        cnt = np.bincount(key, minlength=NPB * NGRP)
        TG = max(TG, int((cnt.max() + P - 1) // P))
        per_core.append((key, rloc, colg, v, cnt))

    NIB = TG * P
    TPB = NGRP * TG

    # --- samples ---
    snew = new_id[nodes]
    sowner = snew // NLOC
    slocal = snew % NLOC
    scnt = np.bincount(sowner, minlength=NCORES)
    NSB = max(1, int((scnt.max() + P - 1) // P))
    NS = NSB * P

    ident_np = np.eye(P, dtype=np.float32)
    wrt_np = np.ascontiguousarray(Wr_w.T).astype(ml_dtypes.bfloat16)
    wt_np = np.ascontiguousarray(W_w.T).astype(ml_dtypes.bfloat16)
    wb_np = np.ascontiguousarray(bil_W[0])           # [256, 256]
    bilb_np = bil_b.reshape(1, 1)

    in_maps = []
    sample_pos = []   # (core, position-in-core-order) per original sample
    for d in range(NCORES):
        key, rloc, colg, v, cnt = per_core[d]
        start = np.concatenate([[0], np.cumsum(cnt)])[:-1]
        off = np.arange(len(key)) - start[key]
        bl = key // NGRP
        grp = key % NGRP
        tl = grp * TG + off // P                      # tile within block
        lane = off % P
        idx_arr = np.zeros((NPB, NGRP, NIB), dtype=np.int16)
        idx_arr[bl, grp, off] = colg
        sva_np = np.zeros((NPB, P, TPB * P), dtype=ml_dtypes.bfloat16)
        sva_np[bl, lane, tl * P + rloc] = v
        # wrapped idx layout: value k -> partition k%16 (x8 replicated), col k//16
        w = idx_arr.reshape(NPB, NGRP, NIB // 16, 16).transpose(0, 1, 3, 2)
        w = np.tile(w, (1, 1, 8, 1))                 # [NPB, NGRP, 128, NIB//16]
        idxg_np = np.ascontiguousarray(
            w.transpose(0, 2, 1, 3).reshape(NPB, P, NGRP * (NIB // 16)))
        # mask of occupied slots
        occ = (old_of_new[d * NLOC:(d + 1) * NLOC] >= 0).astype(np.float32)
        mska_np = np.ascontiguousarray(occ.reshape(NPB, P).T)   # [P, NPB]
        # AXT shards
        olds = old_of_new[d * NLOC:(d + 1) * NLOC]
        valid = olds >= 0
        tmp1 = np.zeros((NLOC, NIN), dtype=np.float32)
        tmp1[valid] = AX1[olds[valid]]
        tmp2 = np.zeros((NLOC, NIN), dtype=np.float32)
        tmp2[valid] = AX2[olds[valid]]
        axt1_np = np.ascontiguousarray(tmp1.T).astype(ml_dtypes.bfloat16)
        axt2_np = np.ascontiguousarray(tmp2.T).astype(ml_dtypes.bfloat16)
        # samples owned by this core
        sp = np.nonzero(sowner == d)[0]
        sample_pos.append(sp)
        sl = np.zeros(NS, dtype=np.int32)
        sl[:len(sp)] = slocal[sp]
        sidx_np = np.ascontiguousarray(sl.reshape(NSB, P).T)    # [P, NSB]
        in_maps.append(dict(
            axt1=axt1_np, axt2=axt2_np, wrt=wrt_np, wt=wt_np, wb=wb_np,
            bilb=bilb_np, ident=ident_np, idxg=idxg_np, sva=sva_np,
            mska=mska_np, sidx=sidx_np))
    return in_maps, (TG, NSB, sample_pos)


def kernel(**inputs) -> np.ndarray:
    in_maps, (TG, NSB, sample_pos) = _prep(inputs)
    key = (TG, NSB)
    if key not in _PROG_CACHE:
        _PROG_CACHE[key] = _build_program(TG, NSB)
    nc = _PROG_CACHE[key]
    last = None
    for _ in range(3):
        try:
            res = run_bass_kernel_spmd(nc, in_maps, core_ids=list(range(NCORES)))
            break
        except Exception as e:   # wedged device -> retry
            last = e
    else:
        raise last
    out = np.zeros((1, 2 * NSAMP), dtype=np.float32)
    for d in range(NCORES):
        sc = res.results[d]["out_sc"]          # [2, NS]
        sp = sample_pos[d]
        out[0, sp] = sc[0, :len(sp)]
        out[0, NSAMP + sp] = sc[1, :len(sp)]
    return out
